# revision 1
# baseline (speedup 1.0000x reference)
"""MultiHeadCrossAttention Trainium2 kernel (8-core SPMD, query-parallel).

Sharding: core c handles batch b=c//4, query rows [1024*(c%4), +1024), all 8
heads.  Each core returns a disjoint [256, 1024] slice of out^T for its batch;
the host gather is a pure concat + transpose.

On-device layout is fully transposed ([channel, position]), matching the raw
[B, C, H, W] input layout, so no transposes are needed anywhere:
  q^T/k^T : [d, pos]   via  lhsT=W^T chunk [c,32|128], rhs=x^T chunk [c, pos]
  scores^T: [kpos, q]  via  lhsT=k^T [32,128] row-tiled 4x, rhs=q^T [32,512]
  exp     : ACT, PSUM->SBUF bf16, FD=1024 (the kernel's critical path:
            256 ACTIVATEs x ~1.0us is the ~260us floor)
  attn@v  : lhsT=[v|1] [128,33], rhs=p^T [128,512], col-tiled 2x (out
            partitions 0-32 / 64-96); the ones column yields softmax
            denominators in rows 32/96 of the accumulating matmuls
  norm    : denominators DMA-gathered into [128,64] tiles, approx-reciprocal,
            DMA back to a [1,NQ] row, gpsimd partition-broadcast, DVE mul
  final   : y^T accumulated in SBUF, one K=32 matmul + DVE add per head

Emission order software-pipelines head h+1's projections under head h's
attention so the ACT engine starts exp'ing within ~10us of kernel start.
All DVE/ACT ops keep in/out on identical partition ranges (walrus verifier
requirement); every cross-partition move rides on DMA or the PE.
"""

import numpy as np
import ml_dtypes

B, C, N, HEADS, D = 2, 256, 4096, 8, 32
NQ = 1024          # queries per core
NCORES = 8
CC = C // 128      # contraction chunks (2)

BF16 = ml_dtypes.bfloat16

_cached = {}
CFG = {"colattn": False, "scores4": True, "interleave": True, "norm": True, "debug": False}


def _build_nc():
    import concourse.bass as bass
    import concourse.bacc as bacc
    import concourse.tile as tile
    import concourse.mybir as mybir
    from contextlib import ExitStack

    fp32 = mybir.dt.float32
    bf16 = mybir.dt.bfloat16
    Exp = mybir.ActivationFunctionType.Exp

    nc = bacc.Bacc("TRN2", target_bir_lowering=False, debug=False,
                   num_devices=NCORES)

    src_d = nc.dram_tensor("src_bf", [C, N], bf16, kind="ExternalInput")
    tgt_d = nc.dram_tensor("tgt_bf", [C, NQ], bf16, kind="ExternalInput")
    tgt8_d = nc.dram_tensor("tgt8", [32, HEADS * NQ], fp32,
                            kind="ExternalInput")
    wq4_d = nc.dram_tensor("wq4", [C, HEADS * 128], bf16, kind="ExternalInput")
    wk_d = nc.dram_tensor("wkT", [C, C], bf16, kind="ExternalInput")
    wv_d = nc.dram_tensor("wvT", [C, C], bf16, kind="ExternalInput")
    wo8_d = nc.dram_tensor("wo8", [32, HEADS * C], bf16, kind="ExternalInput")
    y_d = nc.dram_tensor("yT", [C, NQ], fp32, kind="ExternalOutput")
    if CFG["debug"]:
        dbg_xw0_d = nc.dram_tensor("dbg_xw0", [32, NQ], fp32,
                                   kind="ExternalOutput")
        dbg_rbs0_d = nc.dram_tensor("dbg_rbs0", [32, NQ], fp32,
                                    kind="ExternalOutput")
        dbg_xf0_d = nc.dram_tensor("dbg_xf0", [32, NQ], fp32,
                                   kind="ExternalOutput")

    with tile.TileContext(nc) as tc, ExitStack() as ctx:
        konst = ctx.enter_context(tc.tile_pool(name="konst", bufs=1))
        work = ctx.enter_context(tc.tile_pool(name="work", bufs=1))
        p_pool = ctx.enter_context(tc.tile_pool(name="p", bufs=4))
        sm_pool = ctx.enter_context(tc.tile_pool(name="sm", bufs=2))
        xb_pool = ctx.enter_context(tc.tile_pool(name="xb", bufs=2))
        # PSUM budget (8 banks): ps tiles are [128,1024] (2 banks each),
        # po is [128,1024] when col-tiled else [128,512]
        po_bufs = 1 if CFG["colattn"] else 2
        ps_pool = ctx.enter_context(tc.tile_pool(name="ps", bufs=3, space="PSUM"))
        po_pool = ctx.enter_context(tc.tile_pool(name="po", bufs=po_bufs, space="PSUM"))
        pj_pool = po_pool

        # ---- load inputs (ordered so kproj/vproj deps land first; the DMA
        # stream is HBM-bound ~15us and overlaps the first attention rounds)
        src_sb = konst.tile([128, CC * N], bf16, tag="src")
        tgt_sb = konst.tile([128, CC * NQ], bf16, tag="tgt")
        tgt8_sb = konst.tile([32, HEADS * NQ], fp32, tag="tgt8")
        wq4_sb = konst.tile([128, CC * HEADS * 128], bf16, tag="wq4")
        wk_sb = konst.tile([128, CC * C], bf16, tag="wk")
        wv_sb = konst.tile([128, CC * C], bf16, tag="wv")
        wo8_sb = konst.tile([32, HEADS * C], bf16, tag="wo8")

        def dma_w(w_sb, w_d, eng=None):
            for cc in range(CC):
                (eng or nc.sync).dma_start(w_sb[:, cc * C:(cc + 1) * C],
                                           w_d.ap()[128 * cc:128 * (cc + 1), :])

        def dma_src_half(half, eng=None):
            for cc in range(CC):
                (eng or nc.sync).dma_start(
                    src_sb[:, cc * N + 2048 * half: cc * N + 2048 * (half + 1)],
                    src_d.ap()[128 * cc:128 * (cc + 1),
                               2048 * half:2048 * (half + 1)])

        dma_w(wk_sb, wk_d)
        dma_src_half(0)
        dma_w(wv_sb, wv_d)
        for cc in range(CC):
            nc.sync.dma_start(wq4_sb[:, cc * 1024:(cc + 1) * 1024],
                              wq4_d.ap()[128 * cc:128 * (cc + 1), :])
        for cc in range(CC):
            nc.sync.dma_start(tgt_sb[:, cc * NQ:(cc + 1) * NQ],
                              tgt_d.ap()[128 * cc:128 * (cc + 1), :])
        dma_src_half(1)
        nc.sync.dma_start(tgt8_sb[:], tgt8_d.ap()[:, :])
        nc.sync.dma_start(wo8_sb[:], wo8_d.ap()[:, :])

        # ---- persistent tiles ---------------------------------------------
        kT = [konst.tile([128, 1024], bf16, tag=f"kT{h}", name=f"kT{h}")
              for h in range(HEADS)]
        qT = [konst.tile([128, NQ], bf16, tag=f"qT{h}", name=f"qT{h}")
              for h in range(HEADS)]
        v_sb = konst.tile([128, HEADS * 33 * 32], bf16, tag="v")
        for h in range(HEADS):
            ones_ap = v_sb[:].rearrange("p (h k c) -> p h k c", h=HEADS, k=32)[
                :, h, :, 32:33]
            nc.gpsimd.memset(ones_ap, 1.0)
        xwh = [work.tile([32, NQ], fp32, tag=f"xw{h}", name=f"xw{h}")
               for h in range(HEADS)]
        # softmax denominators, one [32,64] tile per head-PAIR at partition
        # base 0 (custom-DVE ops corrupt at base!=0 on HW): tile[p, f] =
        # sums_flat[64p+f], flat = 1024*(h%2) + q; a/b = col-tile halves
        sums_a = [work.tile([32, 64], fp32, tag=f"sa{i}", name=f"sa{i}")
                  for i in range(4)]
        sums_b = [work.tile([32, 64], fp32, tag=f"sb{i}", name=f"sb{i}")
                  for i in range(4)]
        ssum_p = [work.tile([32, 64], fp32, tag=f"ss{i}", name=f"ss{i}")
                  for i in range(4)]
        rsum_p = [work.tile([32, 64], fp32, tag=f"rs{i}", name=f"rs{i}")
                  for i in range(4)]
        if not CFG["colattn"]:
            for i in range(4):
                nc.gpsimd.memset(sums_b[i][:], 0.0)
        yacc = [work.tile([128, NQ], fp32, tag=f"yacc{t}", name=f"yacc{t}")
                for t in range(CC)]

        v_done = set()

        def vproj(kc):
            if kc in v_done:
                return
            v_done.add(kc)
            ps = pj_pool.tile([128, 512], fp32, tag="po", name=f"psv{kc}")
            for cc in range(CC):
                nc.tensor.matmul(
                    ps[:, 0:256],
                    lhsT=src_sb[:, cc * N + 128 * kc: cc * N + 128 * kc + 128],
                    rhs=wv_sb[:, cc * C:(cc + 1) * C],
                    start=(cc == 0), stop=(cc == CC - 1),
                    tile_position=(0, 0))
            dest = v_sb[:].rearrange("p (h k c) -> p h k c", h=HEADS, k=32)[
                :, :, kc, 0:32]
            nc.vector.tensor_copy(dest, ps[:, 0:256])

        def kqproj_steps(h):
            # k^T folded: strip g (partitions 32g..) holds kpos block b=4jj+g
            # at cols [512jj, +512); kc for 128-col slice m: 16*(m//4)+4g+(m%4)
            # Returned as small closures so callers can interleave them under
            # attention rounds (a single burst stalls ACT at head boundaries).
            steps = []
            state = {}

            def k_step(jj):
                def run():
                    ps = pj_pool.tile([128, 512], fp32, tag="po",
                                      name=f"psk{h}_{jj}")
                    for cc in range(CC):
                        for g in range(4):
                            blk = 4 * jj + g
                            nc.tensor.matmul(
                                ps[32 * g:32 * g + 32, 0:512],
                                lhsT=wk_sb[:, cc * C + 32 * h: cc * C + 32 * h + 32],
                                rhs=src_sb[:, cc * N + 512 * blk: cc * N + 512 * blk + 512],
                                start=(cc == 0), stop=(cc == CC - 1),
                                tile_position=(0, 32 * g))
                    nc.vector.tensor_copy(
                        kT[h][:, 512 * jj:512 * jj + 512], ps[:, 0:512])
                return run

            def q_step(qb):
                def run():
                    ps = pj_pool.tile([128, 512], fp32, tag="po",
                                      name=f"psq{h}_{qb}")
                    for cc in range(CC):
                        nc.tensor.matmul(
                            ps[:, 0:512],
                            lhsT=wq4_sb[:, cc * 1024 + 128 * h: cc * 1024 + 128 * h + 128],
                            rhs=tgt_sb[:, cc * NQ + 512 * qb: cc * NQ + 512 * qb + 512],
                            start=(cc == 0), stop=(cc == CC - 1),
                            tile_position=(0, 0))
                    nc.vector.tensor_copy(qT[h][:, 512 * qb:512 * qb + 512],
                                          ps[:, 0:512])
                return run

            for jj in range(2):
                steps.append(k_step(jj))
            for qb in range(NQ // 512):
                steps.append(q_step(qb))
            return steps

        def kqproj(h):
            for st in kqproj_steps(h):
                st()

        def attn_unit(h, qb, feed=()):
            feed = list(feed)
            """One (head, 512-query-block) attention unit: 8 superrounds of
            4 k-chunks; scores row-tiled 4x across partition strips, attn@v
            col-tiled 2x (strips 0/1 -> out partitions 0-32, strips 2/3 ->
            64-96)."""
            po = po_pool.tile([128, 1024 if CFG["colattn"] else 512], fp32,
                              tag="po", name=f"po{h}_{qb}")
            rounds = []
            if CFG["scores4"]:
                rounds = [((0, 1, 2, 3), m) for m in range(8)]
            else:
                rounds = [((0, 1) if r % 2 == 0 else (2, 3), r // 2)
                          for r in range(16)]
            first = True
            for ri, (strips, m) in enumerate(rounds):
                last = ri == len(rounds) - 1
                if h == 0 and qb == 0:
                    for g in strips:
                        vproj(16 * (m // 4) + 4 * g + (m % 4))
                if feed:
                    feed.pop(0)()
                ntile = len(strips) // 2
                pss = [ps_pool.tile([128, 1024], fp32, tag="ps",
                                    name=f"ps{h}_{qb}_{ri}_{i}")
                       for i in range(ntile)]
                for gi, g in enumerate(strips):
                    nc.tensor.matmul(
                        pss[gi // 2][:, 512 * (gi % 2):512 * (gi % 2) + 512],
                        lhsT=kT[h][32 * g:32 * g + 32, 128 * m:128 * m + 128],
                        rhs=qT[h][32 * g:32 * g + 32, 512 * qb:512 * qb + 512],
                        start=True, stop=True,
                        tile_position=(32 * g, 0))
                pbs = []
                for i in range(ntile):
                    p_sb = p_pool.tile([128, 1024], bf16, tag="p",
                                       name=f"p{h}_{qb}_{ri}_{i}")
                    nc.scalar.activation(p_sb[:], pss[i][:, 0:1024], Exp)
                    pbs.append(p_sb)
                for gi, g in enumerate(strips):
                    kc = 16 * (m // 4) + 4 * g + (m % 4)
                    if CFG["colattn"]:
                        co = 64 * (gi % 2)
                        fo = 512 * (gi % 2)
                        st = first and gi < 2
                        sp = last and gi >= len(strips) - 2
                    else:
                        co, fo = 0, 0
                        st = first and gi == 0
                        sp = last and gi == len(strips) - 1
                    nc.tensor.matmul(
                        po[co:co + 33, fo:fo + 512],
                        lhsT=v_sb[:, 1056 * h + 33 * kc: 1056 * h + 33 * kc + 33],
                        rhs=pbs[gi // 2][:, 512 * (gi % 2):512 * (gi % 2) + 512],
                        start=st, stop=sp,
                        tile_position=(0, co))
                first = False
            for st in feed:
                st()
            # drain A (partitions 0-32) and B (64-96); merge B via DMA hop
            hp, prow = h // 2, 16 * (h % 2) + 8 * qb
            nc.vector.tensor_copy(xwh[h][:, 512 * qb:512 * qb + 512],
                                  po[0:32, 0:512])
            stmp = sm_pool.tile([97, 512], fp32, tag="stmp", name=f"st{h}{qb}")
            nc.vector.tensor_copy(stmp[32:33, 0:512], po[32:33, 0:512])
            nc.sync.dma_start(sums_a[hp][prow:prow + 8, 0:64],
                              stmp[32:33, 0:512])
            if CFG["colattn"]:
                xb64 = xb_pool.tile([96, 512], fp32, tag="xb64",
                                    name=f"xb64_{h}{qb}")
                nc.vector.tensor_copy(xb64[64:96, 0:512], po[64:96, 512:1024])
                xb0 = xb_pool.tile([32, 512], fp32, tag="xb0", name=f"xb0_{h}{qb}")
                nc.sync.dma_start(xb0[:], xb64[64:96, 0:512])
                nc.vector.tensor_add(xwh[h][:, 512 * qb:512 * qb + 512],
                                     xwh[h][:, 512 * qb:512 * qb + 512], xb0[:])
                nc.vector.tensor_copy(stmp[96:97, 0:512], po[96:97, 512:1024])
                nc.sync.dma_start(sums_b[hp][prow:prow + 8, 0:64],
                                  stmp[96:97, 0:512])


        def recip_pair(h):
            hp = h // 2
            nc.vector.tensor_add(ssum_p[hp][:], sums_a[hp][:], sums_b[hp][:])
            nc.vector.reciprocal_approx_fast(rsum_p[hp][:], ssum_p[hp][:])

        def normalize(h):
            """Broadcast 1/denominator, scale w^T, add residual, accumulate
            this head's contribution to y^T."""
            rrow = sm_pool.tile([1, NQ], fp32, tag="rrow", name=f"rr{h}")
            nc.sync.dma_start(rrow[:],
                              rsum_p[h // 2][16 * (h % 2):16 * (h % 2) + 16,
                                             0:64])
            rbs = sm_pool.tile([32, NQ], fp32, tag="rbs", name=f"rb{h}")
            nc.gpsimd.partition_broadcast(rbs[:], rrow[:])
            if CFG["debug"] and h == 0:
                nc.sync.dma_start(dbg_xw0_d.ap()[:, :], xwh[h][:])
                nc.sync.dma_start(dbg_rbs0_d.ap()[:, :], rbs[:])
            nc.vector.tensor_mul(xwh[h][:], xwh[h][:], rbs[:])
            xfh = xb_pool.tile([32, NQ], bf16, tag="xfh", name=f"xf{h}")
            nc.vector.tensor_add(xfh[:], xwh[h][:],
                                 tgt8_sb[:, NQ * h:NQ * (h + 1)])
            if CFG["debug"] and h == 0:
                xf32 = xb_pool.tile([32, NQ], fp32, tag="xf32", name="xf32d")
                nc.vector.tensor_copy(xf32[:], xfh[:])
                nc.sync.dma_start(dbg_xf0_d.ap()[:, :], xf32[:])
            for dc in range(CC):
                for qb in range(NQ // 512):
                    ps = pj_pool.tile([128, 512], fp32, tag="po",
                                      name=f"py{h}_{dc}_{qb}")
                    nc.tensor.matmul(
                        ps[:, 0:512],
                        lhsT=wo8_sb[:, C * h + 128 * dc: C * h + 128 * dc + 128],
                        rhs=xfh[:, 512 * qb:512 * qb + 512],
                        start=True, stop=True, tile_position=(0, 0))
                    if h == 0:
                        nc.vector.tensor_copy(
                            yacc[dc][:, 512 * qb:512 * qb + 512], ps[:, 0:512])
                    else:
                        nc.vector.tensor_add(
                            yacc[dc][:, 512 * qb:512 * qb + 512],
                            yacc[dc][:, 512 * qb:512 * qb + 512], ps[:, 0:512])

        # ---- emission: software-pipeline projections under attention ------
        if CFG["interleave"]:
            kqproj(0)
            for h in range(HEADS):
                steps = kqproj_steps(h + 1) if h + 1 < HEADS else []
                if h == 0:
                    attn_unit(h, 0, feed=steps[:2])
                    attn_unit(h, 1, feed=steps[2:])
                else:
                    attn_unit(h, 0, feed=steps)
                    attn_unit(h, 1)
                # normalization for pair p emits under pair p+1's attention
                # so its DMA/gpsimd latency never blocks the DVE queue
                if h % 2 == 1:
                    recip_pair(h)
                if h % 2 == 0 and h >= 2:
                    normalize(h - 2)
                    normalize(h - 1)
            normalize(HEADS - 2)
            normalize(HEADS - 1)
        else:
            for kc in range(32):
                vproj(kc)
            for h in range(HEADS):
                kqproj(h)
            for h in range(HEADS):
                for qb in range(NQ // 512):
                    attn_unit(h, qb)
            for h in range(HEADS):
                if h % 2 == 1:
                    recip_pair(h)
                    normalize(h - 1)
                    normalize(h)

        for dc in range(CC):
            nc.sync.dma_start(y_d.ap()[128 * dc:128 * (dc + 1), :],
                              yacc[dc][:])

    nc.compile()
    return nc


def _prep_core_inputs(core, tgt, src, Wq, Wk, Wv, Wo):
    b, qoff = core // 4, NQ * (core % 4)
    srcT = src[b].reshape(C, N)
    tgtT = tgt[b].reshape(C, N)[:, qoff:qoff + NQ]
    scale = 1.0 / np.sqrt(np.float32(D))
    wqT = (Wq * scale).T.astype(BF16)
    wq4 = np.empty((C, HEADS * 128), dtype=BF16)
    for h in range(HEADS):
        wq4[:, 128 * h:128 * (h + 1)] = np.tile(wqT[:, 32 * h:32 * h + 32],
                                                (1, 4))
    # per-head row blocks of tgt^T / Wo^T laid side by side at partitions 0-31
    tgt8 = np.empty((32, HEADS * NQ), dtype=np.float32)
    woT = Wo.T.astype(np.float32)
    wo8 = np.empty((32, HEADS * C), dtype=BF16)
    for h in range(HEADS):
        tgt8[:, NQ * h:NQ * (h + 1)] = tgtT[32 * h:32 * h + 32, :]
        wo8[:, C * h:C * (h + 1)] = woT[32 * h:32 * h + 32, :].astype(BF16)
    return {
        "src_bf": np.ascontiguousarray(srcT).astype(BF16),
        "tgt_bf": np.ascontiguousarray(tgtT).astype(BF16),
        "tgt8": tgt8,
        "wq4": wq4,
        "wkT": np.ascontiguousarray(Wk.T).astype(BF16),
        "wvT": np.ascontiguousarray(Wv.T).astype(BF16),
        "wo8": wo8,
    }


def kernel(tgt, src, Wq, Wk, Wv, Wo, _want_results=False):
    from concourse.bass_utils import run_bass_kernel_spmd

    tgt = np.asarray(tgt, dtype=np.float32)
    src = np.asarray(src, dtype=np.float32)
    Wq = np.asarray(Wq, dtype=np.float32)
    Wk = np.asarray(Wk, dtype=np.float32)
    Wv = np.asarray(Wv, dtype=np.float32)
    Wo = np.asarray(Wo, dtype=np.float32)

    if "nc" not in _cached:
        _cached["nc"] = _build_nc()
    nc = _cached["nc"]

    in_maps = [_prep_core_inputs(c, tgt, src, Wq, Wk, Wv, Wo)
               for c in range(NCORES)]
    res = run_bass_kernel_spmd(nc, in_maps, core_ids=list(range(NCORES)))

    out = np.empty((B, N, C), dtype=np.float32)
    for c in range(NCORES):
        b, qoff = c // 4, NQ * (c % 4)
        out[b, qoff:qoff + NQ, :] = res.results[c]["yT"].T
    if _want_results:
        return out, res
    return out

